# revision 16
# baseline (speedup 1.0000x reference)
"""Causal attention (B=4, L=2048, d_model=1024, d_k=d_v=128) on 8 TRN2 NeuronCores.

Sharding (SPMD — one program, per-core data):
  core c -> batch b = c//2, parity par = c%2.
  Core handles q-blocks j = 2k+par for slot k in 0..7 (128 rows each).
  Slot k covers key blocks [0, 2k+2) — a uniform instruction stream across
  cores; the causal boundary (which depends on parity) is applied with a
  per-core additive mask fed as data.  Every core projects K/V for the full
  2048 rows of its batch (KV compute duplicated within a pair; no
  collectives).

Within a core (all matmuls contract on the partition dim):
  - X is fed pre-transposed (X^T: d_model x L) plus a packed copy of this
    core's query rows (X_q: d_model x 1024).
  - Projections are weight-stationary: for each W chunk, 2-4 column groups
    of 512 accumulate in separate PSUM banks.
  - Scores are computed TRANSPOSED: S^T[key, q] = K^T_blk.T @ Q^T, one
    N<=512 matmul per (key block, slot group of 4).  exp() then writes A^T
    straight to SBUF (bf16) — no PE transposes or extra copies for A.
  - V is augmented with a ones column; Z_aug = A^T.T @ [V | 1] yields the
    softmax denominator in column 128 for free.  Softmax skips the row-max
    subtraction (scores here are bounded ~|12|; exp is safe in f32).
"""

import os
import sys

sys.path.insert(0, "/opt/trn_rl_repo")
sys.path.insert(0, "/opt/trn_rl_repo/concourse")

import ml_dtypes
import numpy as np

import concourse.bass as bass  # noqa: F401
import concourse.mybir as mybir
import concourse.tile as tile
from concourse import bacc
from concourse.bass_utils import run_bass_kernel_spmd
from concourse.masks import make_identity

B, L, DM, DK, DV = 4, 2048, 1024, 128, 128
NB = L // 128   # 16 key blocks per batch
SLOTS = 8       # q-blocks per core
NCH = DM // 128  # 8 d_model chunks
SCALE = float(DK) ** -0.5
MASKVAL = -1e9

COMPUTE = os.environ.get("ATTN_COMPUTE", "bf16")  # "bf16" | "f32"

F32 = mybir.dt.float32


def _cdt():
    return mybir.dt.bfloat16 if COMPUTE == "bf16" else mybir.dt.float32


def _np_cdt():
    return ml_dtypes.bfloat16 if COMPUTE == "bf16" else np.float32


def build_nc():
    cdt = _cdt()
    nc = bacc.Bacc()

    xt_ext = nc.declare_dram_parameter("xt", [DM, L], cdt, isOutput=False)
    xq_ext = nc.declare_dram_parameter("xq", [DM, SLOTS * 128], cdt, isOutput=False)
    wq_ext = nc.declare_dram_parameter("wq", [DM, DK], cdt, isOutput=False)
    wk_ext = nc.declare_dram_parameter("wk", [DM, DK], cdt, isOutput=False)
    wv_ext = nc.declare_dram_parameter("wv", [DM, DK], cdt, isOutput=False)
    # transposed boundary masks: [key 128, 2*128 q] — col block 0 applied at
    # key block 2k, col block 1 at key block 2k+1 (for slot k)
    mask_ext = nc.declare_dram_parameter("maskT", [128, 256], F32, isOutput=False)
    out_ext = nc.declare_dram_parameter("out", [SLOTS * 128, DV], F32, isOutput=True)

    with tile.TileContext(nc) as tc:
        with (
            tc.tile_pool(name="persist", bufs=1) as persist,
            tc.tile_pool(name="mm_ps", bufs=4, space="PSUM") as mm_ps,
            tc.tile_pool(name="tp_ps", bufs=2, space="PSUM") as tp_ps,
            tc.tile_pool(name="z_ps", bufs=2, space="PSUM") as z_ps,
            tc.tile_pool(name="work", bufs=2) as work,
        ):
            # ---- constants / inputs ----
            ident = persist.tile([128, 128], cdt, tag="ident")
            make_identity(nc, ident)

            # small inputs first so projections can start while X streams in
            w_sb = {}
            for name, ext in (("wq", wq_ext), ("wk", wk_ext), ("wv", wv_ext)):
                t = persist.tile([128, NCH, 128], cdt, tag=name)
                nc.sync.dma_start(
                    out=t[:], in_=ext.rearrange("(nc p) d -> p nc d", p=128)
                )
                w_sb[name] = t

            mask_sb = persist.tile([128, 256], F32, tag="mask")
            nc.sync.dma_start(out=mask_sb[:], in_=mask_ext[:])

            # X^T / X_q in sequence-major halves: all 8 d_model chunks of the
            # first half of the columns arrive before any of the second half,
            # so K^T/V^T groups 0-1 (and scores on key blocks 0-7) can start
            # while the rest of X streams in.
            xq_t = persist.tile([128, NCH, SLOTS * 128], cdt, tag="xq")
            xt_t = persist.tile([128, NCH, L], cdt, tag="xt")
            xq_r = xq_ext.rearrange("(c p) l -> p c l", p=128)
            xt_r = xt_ext.rearrange("(c p) l -> p c l", p=128)
            # xq: one DMA per d_model chunk (QT chases these); xt: one DMA
            # per 512-column piece (K^T/V^T group g chases piece g), issued
            # on the Scalar HWDGE queue so descriptor generation overlaps
            # the Sync queue's.
            for c in range(NCH):
                nc.sync.dma_start(out=xq_t[:, c, :], in_=xq_r[:, c, :])
            for p in range(4):
                nc.scalar.dma_start(
                    out=xt_t[:, :, p * 512:(p + 1) * 512],
                    in_=xt_r[:, :, p * 512:(p + 1) * 512],
                )
            xq = [xq_t[:, c, :] for c in range(NCH)]
            xt = [xt_t[:, c, :] for c in range(NCH)]

            # ---- projections (weight-stationary; groups accumulate in
            # separate PSUM banks) ----
            qt_sb = persist.tile([128, SLOTS * 128], cdt, tag="qt")
            kt_sb = persist.tile([128, L], cdt, tag="kt")
            vt_sb = persist.tile([128, L], cdt, tag="vt")
            v_aug = persist.tile([128, NB, DV + 1], cdt, tag="vaug")
            nc.vector.memset(v_aug[:, :, DV:DV + 1], 1.0)
            at_g = [
                persist.tile([128, NB, 512], cdt, tag=f"at{g}", name=f"at{g}")
                for g in range(2)
            ]

            def proj(name, src, dst, scale, gs):
                w = w_sb[name]
                for g in gs:
                    ps = mm_ps.tile([128, 512], F32, tag="mm", name=f"pj{g}")
                    for c in range(NCH):
                        nc.tensor.matmul(
                            ps[:],
                            w[:, c, :],
                            src[c][:, g * 512:(g + 1) * 512],
                            start=(c == 0),
                            stop=(c == NCH - 1),
                        )
                    dslice = dst[:, g * 512:(g + 1) * 512]
                    if scale is not None:
                        nc.scalar.activation(
                            dslice, ps[:],
                            mybir.ActivationFunctionType.Copy,
                            bias=0.0, scale=scale,
                        )
                    elif g % 2 == 0:
                        nc.scalar.copy(dslice, ps[:])
                    else:
                        nc.vector.tensor_copy(dslice, ps[:])

            # Q^T for all slots first (xq is the first big DMA to land)
            proj("wq", xq, qt_sb, SCALE, [0, 1])

            for h in range(2):
                # K^T/V^T for this half of the key columns
                proj("wk", xt, kt_sb, None, [2 * h, 2 * h + 1])
                proj("wv", xt, vt_sb, None, [2 * h, 2 * h + 1])

                # V natural (+ ones col already memset)
                for kb in range(h * 8, h * 8 + 8):
                    vps = tp_ps.tile([128, 128], cdt, tag="tp")
                    nc.tensor.transpose(
                        vps[:], vt_sb[:, kb * 128:(kb + 1) * 128], ident[:]
                    )
                    dst = v_aug[:, kb, 0:DV]
                    if kb % 2 == 0:
                        nc.vector.tensor_copy(dst, vps[:])
                    else:
                        nc.scalar.copy(dst, vps[:])

                # scores + exp for key blocks of this half
                for kb in range(h * 8, h * 8 + 8):
                    m = kb // 2          # first active slot
                    for g in range(2):
                        lo = max(m, 4 * g)
                        if lo > 4 * g + 3:
                            continue
                        a = lo - 4 * g
                        st = mm_ps.tile([128, 512], F32, tag="mm")
                        nc.tensor.matmul(
                            st[:, a * 128:512],
                            kt_sb[:, kb * 128:(kb + 1) * 128],
                            qt_sb[:, lo * 128:(4 * g + 4) * 128],
                            start=True, stop=True,
                            skip_group_check=True,
                        )
                        ks = kb // 2
                        if 4 * g <= ks <= 4 * g + 3:
                            qoff = (ks - 4 * g) * 128
                            nc.vector.tensor_add(
                                st[:, qoff:qoff + 128],
                                st[:, qoff:qoff + 128],
                                mask_sb[:, (kb % 2) * 128:(kb % 2 + 1) * 128],
                            )
                        nc.scalar.activation(
                            at_g[g][:, kb, a * 128:512],
                            st[:, a * 128:512],
                            mybir.ActivationFunctionType.Exp,
                            bias=0.0, scale=1.0,
                        )

                # AV for the slots that completed with this half
                for k in range(h * 4, h * 4 + 4):
                    nkb = 2 * k + 2
                    g, q = k // 4, (k % 4) * 128
                    zp = z_ps.tile([128, DV + 1], F32, tag="z")
                    for kb in range(nkb):
                        nc.tensor.matmul(
                            zp[:],
                            at_g[g][:, kb, q:q + 128],
                            v_aug[:, kb, :],
                            start=(kb == 0),
                            stop=(kb == nkb - 1),
                        )
                    rcp = work.tile([128, 1], F32, tag="rcp")
                    nc.vector.reciprocal(rcp[:], zp[:, DV:DV + 1])
                    z_sb = work.tile([128, DV], F32, tag="zout")
                    nc.vector.tensor_scalar_mul(z_sb[:], zp[:, 0:DV], rcp[:])
                    nc.sync.dma_start(
                        out=out_ext[k * 128:(k + 1) * 128, :], in_=z_sb[:]
                    )

    nc.finalize()
    return nc


_NC = None


def _get_nc():
    global _NC
    if _NC is None:
        _NC = build_nc()
    return _NC


def _make_masks():
    p = np.arange(128)[:, None]   # key (partition)
    q = np.arange(128)[None, :]   # query (free)
    triT = np.where(p <= q, 0.0, MASKVAL).astype(np.float32)
    full = np.full((128, 128), MASKVAL, np.float32)
    zero = np.zeros((128, 128), np.float32)
    mask_even = np.concatenate([triT, full], axis=1)
    mask_odd = np.concatenate([zero, triT], axis=1)
    return mask_even, mask_odd


def kernel(X, W_Q, W_K, W_V):
    X = np.asarray(X, np.float32)
    W_Q = np.asarray(W_Q, np.float32)
    W_K = np.asarray(W_K, np.float32)
    W_V = np.asarray(W_V, np.float32)

    nc = _get_nc()
    npdt = _np_cdt()
    mask_even, mask_odd = _make_masks()

    wq = W_Q.astype(npdt)
    wk = W_K.astype(npdt)
    wv = W_V.astype(npdt)

    in_maps = []
    for c in range(8):
        b, par = c // 2, c % 2
        xt_np = np.ascontiguousarray(X[b].T).astype(npdt)
        qcols = np.concatenate(
            [np.arange((2 * k + par) * 128, (2 * k + par + 1) * 128)
             for k in range(SLOTS)]
        )
        in_maps.append({
            "xt": xt_np,
            "xq": np.ascontiguousarray(xt_np[:, qcols]),
            "wq": wq, "wk": wk, "wv": wv,
            "maskT": mask_odd if par else mask_even,
        })

    res = run_bass_kernel_spmd(nc, in_maps, list(range(8)))

    Z = np.zeros((B, L, DV), np.float32)
    for c in range(8):
        b, par = c // 2, c % 2
        o = res.results[c]["out"]
        for k in range(SLOTS):
            j = 2 * k + par
            Z[b, j * 128:(j + 1) * 128, :] = o[k * 128:(k + 1) * 128, :]
    return Z
